# revision 46
# baseline (speedup 1.0000x reference)
"""AttentionSubsample kernel for 8 trn2 NeuronCores.

Sharding: head-parallel (8 heads -> 8 cores), each core handles its head for
both batches through attn@v + hardswish; final projection is sharded by
output channels after a per-chunk AllGather of the per-head attention
outputs.

Key tricks:
- All matmuls in bf16 with fp32 PSUM accumulation.
- S^T layout for the attention matrix (k on partitions, q on free) so both
  QK^T and attn@V are natural matmuls (no transposes of big tensors).
- The relative-position bias is factored out of the softmax numerator:
  exp(qk*scale + b) = exp(qk*scale) * exp(b); exp(b) is a small host-side
  table, expanded to a full (padded-k, q) bf16 tensor per head, streamed from
  HBM and multiplied in on the vector engine (2x bf16 mode). Padded k rows
  get exp(b)=0 which also kills them in the softmax denominator.
- Softmax denominator comes free from the attn@V matmul by appending a ones
  column to V (33rd stationary column).
- BatchNorms (training mode) computed on-device via bn_stats/bn_aggr over the
  full (B*N) token range; affine folded into per-partition tensor_scalars.
- The ACT engine's activation table is loaded exactly once (Exp): the BN
  1/sqrt(var+eps) runs on DVE via the quake-rsqrt bit trick + Newton, and
  PSUM drains use scalar.copy (Copy lives in the same table set as Exp).
- v enters the attention RAW; since softmax weights sum to one and BN is
  affine, v's BN is folded into the drain epilogue as s_v*x + t_v.
- attn@V matmuls are emitted one k-group behind QK^T so the in-order PE queue
  always has QK work while the bias-multiply catches up; each chunk's
  softmax-divide/hardswish is deferred into the next chunk's g-loop so the
  DVE never stalls the ACT exp pipeline at chunk boundaries.
- q/kv projections interleave per batch behind a chunked x DMA; the q
  projection reads the x tile with a stride-2 access pattern (no separate
  subsampled host tensor); bn_stats follows each ACT drain; PSUM chunks
  alternate between two pools for a 4-deep ring.
- 1/denominator is broadcast across partitions on the idle gpsimd engine.
- The post-projection buffer and output are bf16 (converted to f32 on
  host); the final BN scale runs in 4x DVE mode, split in halves so the
  store DMA overlaps the second half; BN stats read fp32 from PSUM.
- Small-op chains (Newton rsqrt, softmax drain epilogue) use fused
  scalar_tensor_tensor ops to cut per-instruction dispatch overhead.
"""

import numpy as np
import ml_dtypes

import concourse.bass as bass
import concourse.mybir as mybir
import concourse.tile as tile
from concourse import bacc
from contextlib import ExitStack
from concourse.bass_utils import run_bass_kernel_spmd

BF16 = mybir.dt.bfloat16
F32 = mybir.dt.float32
I32 = mybir.dt.int32
bf16 = ml_dtypes.bfloat16

B = 2
ROW, COL = 63, 84
ROW_, COL_ = 32, 42
N = ROW * COL            # 5292 kv tokens
NQ = ROW_ * COL_         # 1344 q tokens
NPAD = 5376              # 42*128 padded kv tokens
KT = NPAD // 128         # 42 k-tiles
QC = 448                 # q chunk
NQC = NQ // QC           # 3
CIN = 256
H = 8
KD = 16
DV = 32
HKV = KD + DV            # 48 per-head kv channels
KVP = 64                 # padded kv rows: k at 0:16, v at 32:64 (32-aligned)
OC = 64                  # per-core slice of the 512 output channels
GRP = 3                  # k-tiles per exp group
NGRP = KT // GRP         # 14
EPS = 1e-5
SCALE = KD ** -0.5
NCORES = 8

TCH = 448                # kv-proj token chunk
NT_KV = NPAD // TCH      # 12
XQ = 2                   # x DMA split per (b, c)

LAST_EXEC_NS = None
_prog_cache = {}


def _build_program():
    nc = bacc.Bacc(num_devices=NCORES)

    xT = nc.dram_tensor("xT", [B, 2, 128, NPAD], BF16, kind="ExternalInput")
    wAll = nc.dram_tensor("wAll", [2, 128, KVP + KD + OC], BF16,
                          kind="ExternalInput")
    gbAll = nc.dram_tensor("gbAll", [KVP, 6], F32, kind="ExternalInput")
    ebT = nc.dram_tensor("ebT", [NQC, NGRP, 128, GRP * QC], BF16,
                         kind="ExternalInput")
    yT = nc.dram_tensor("yT", [OC, B * NQ], BF16, kind="ExternalOutput")

    with ExitStack() as ctx:
        tc = ctx.enter_context(tile.TileContext(nc))
        const = ctx.enter_context(tc.tile_pool(name="const", bufs=1))
        big = ctx.enter_context(tc.tile_pool(name="big", bufs=1))
        vtp = ctx.enter_context(tc.tile_pool(name="vtp", bufs=2))
        spool = ctx.enter_context(tc.tile_pool(name="spool", bufs=4))
        ebpool = ctx.enter_context(tc.tile_pool(name="ebpool", bufs=4))
        small = ctx.enter_context(tc.tile_pool(name="small", bufs=4))
        rspool = ctx.enter_context(tc.tile_pool(name="rspool", bufs=2))
        drain = ctx.enter_context(tc.tile_pool(name="drain", bufs=3))
        psA = ctx.enter_context(tc.tile_pool(name="psA", bufs=2, space="PSUM"))
        psB = ctx.enter_context(tc.tile_pool(name="psB", bufs=2, space="PSUM"))
        dram = ctx.enter_context(tc.tile_pool(name="dram", bufs=4, space="DRAM"))

        mult = mybir.AluOpType.mult
        add = mybir.AluOpType.add
        amin = mybir.AluOpType.min
        lshr = mybir.AluOpType.logical_shift_right
        Act = mybir.ActivationFunctionType

        # ------------------------- load inputs -------------------------
        # Small weight/affine tensors first so the projections never wait on
        # them; x streamed in (b, c, quarter) chunks so compute follows the
        # transfer.
        wall_sb = const.tile([128, 2, KVP + KD + OC], BF16, tag="wall")
        gb_sb = const.tile([KVP, 6], F32, tag="gball")
        nc.sync.dma_start(out=wall_sb,
                          in_=wAll.rearrange("c p o -> p c o"))
        nc.sync.dma_start(out=gb_sb, in_=gbAll[:, :])
        wkv_sb = wall_sb[:, :, 0:KVP]
        wq_sb = wall_sb[:, :, KVP:KVP + KD]
        wp_sb = wall_sb[:, :, KVP + KD:KVP + KD + OC]
        kvgb_sb = gb_sb[:, 0:2]
        qgb_sb = gb_sb[0:KD, 2:4]
        pgb_sb = gb_sb[:, 4:6]
        ones1_t = const.tile([1, DV], F32, tag="ones1")
        nc.vector.memset(ones1_t, 1.0)

        xt_sb = big.tile([128, B, 2, NPAD], BF16, tag="xt")
        XCH = NPAD // XQ
        for b in range(B):
            for x4 in range(XQ):
                for c in range(2):
                    nc.sync.dma_start(
                        out=xt_sb[:, b, c, bass.ts(x4, XCH)],
                        in_=xT[b, c, :, bass.ts(x4, XCH)])

        # ------------------------- rsqrt helper -------------------------
        # s = g / sqrt(var + eps), t = beta - mu * s, all on DVE (quake
        # bit-trick + 3 Newton steps) so the ACT engine never needs the Sqrt
        # table (keeps Exp resident the whole kernel).
        def bn_scale_shift(mv, gb, P, name):
            z = small.tile([P, 1], F32, tag=f"z_{name}")
            w = small.tile([P, 1], I32, tag=f"w_{name}")
            t3 = small.tile([P, 1], F32, tag=f"t3_{name}")
            s = small.tile([P, 1], F32, tag=f"s_{name}")
            t = small.tile([P, 1], F32, tag=f"t_{name}")
            nc.vector.tensor_scalar(out=z, in0=mv[:, 1:2], scalar1=EPS,
                                    scalar2=None, op0=add)
            nc.vector.tensor_scalar(out=w, in0=z.bitcast(I32), scalar1=1,
                                    scalar2=None, op0=lshr)
            nc.vector.tensor_scalar(out=w, in0=w, scalar1=-1,
                                    scalar2=0x5f3759df, op0=mult, op1=add)
            y = w.bitcast(F32)
            for _ in range(2):
                nc.vector.tensor_mul(t3, y, y)
                nc.vector.scalar_tensor_tensor(out=t3, in0=t3, scalar=-0.5,
                                               in1=z, op0=mult, op1=mult)
                nc.vector.scalar_tensor_tensor(out=y, in0=t3, scalar=1.5,
                                               in1=y, op0=add, op1=mult)
            nc.vector.tensor_mul(s, y, gb[:, 0:1])
            nc.vector.scalar_tensor_tensor(out=t, in0=mv[:, 0:1], scalar=-1.0,
                                           in1=s, op0=mult, op1=mult)
            nc.vector.tensor_add(t, t, gb[:, 1:2])
            return s, t

        # ------------- q + kv projections (interleaved per batch) -------------
        # q proj is tiny and its BN chain (stats/aggr/rsqrt/norm) finishes
        # during the kv window, so after the kv aggregate only the kv rsqrt +
        # kT norms stand between the last drain and the attention loop.
        # kv stats blocks align with drain chunks; the last trims the 84
        # padded tokens (5292 = 11*448 + 364). Drains run on ACT (Copy shares
        # the Exp table set); bn_stats follows on DVE from the drained bf16.
        y_kv = big.tile([KVP, B, NPAD], BF16, tag="ykv")
        st_kv = small.tile([KVP, 2 * NT_KV, 6], F32, tag="st_kv")
        v_aug = big.tile([128, B, KT, DV + 1], BF16, tag="vaug")
        xq_view = xt_sb.rearrange("p b c (r w) -> p b c r w", w=COL)
        y_q = big.tile([KD, B, NQ], BF16, tag="yq")
        QRC = 8                    # q rows per proj chunk
        NT_Q = ROW_ // QRC         # 4 chunks of 336 q tokens
        QCH = QRC * COL_           # 336
        st_q = small.tile([KD, 2 * NT_Q, 6], F32, tag="st_q")
        for b in range(B):
            for t in range(NT_Q):
                pool = psA if t % 2 == 0 else psB
                ps = pool.tile([KD, QCH], F32,
                               tag="qk" if t % 2 == 0 else "ps_small")
                for c in range(2):
                    # moving AP: q rows stride 2 in r, cols stride 2 in w
                    qv = xq_view[:, b, c,
                                 2 * t * QRC:2 * (t + 1) * QRC:2,
                                 0:2 * COL_:2]
                    nc.tensor.matmul(ps, wq_sb[:, c, :], qv,
                                     start=(c == 0), stop=(c == 1))
                nc.scalar.copy(out=y_q[:, b, bass.ts(t, QCH)], in_=ps)
                nc.vector.bn_stats(out=st_q[:, b * NT_Q + t, :],
                                   in_=y_q[:, b, bass.ts(t, QCH)])
            for t in range(NT_KV):
                pool = psA if t % 2 == 0 else psB
                ps = pool.tile([KVP, TCH], F32,
                               tag="qk" if t % 2 == 0 else "ps_small")
                for c in range(2):
                    nc.tensor.matmul(ps, wkv_sb[:, c, :],
                                     xt_sb[:, b, c, bass.ts(t, TCH)],
                                     start=(c == 0), stop=(c == 1))
                nc.scalar.copy(out=y_kv[:, b, bass.ts(t, TCH)], in_=ps)
                nv = TCH if t < NT_KV - 1 else (N - (NT_KV - 1) * TCH)
                nc.vector.bn_stats(out=st_kv[:, b * NT_KV + t, :],
                                   in_=y_kv[:, b, bass.ds(t * TCH, nv)])
            # v goes into the attention RAW (pad rows are zero); its BN
            # affine is folded into the drain epilogue (exact: BN is affine,
            # and the softmax weights sum to one).
            vtd = vtp.tile([128, KT, DV], BF16, tag="vtd")
            nc.sync.dma_start_transpose(out=vtd, in_=y_kv[32:KVP, b, :])
            nc.vector.tensor_copy(v_aug[:, b, :, 0:DV], vtd)
            nc.vector.memset(v_aug[:, b, :, DV:DV + 1], 1.0)

        kT = big.tile([KD, B, NPAD], BF16, tag="kT")
        qT = big.tile([KD, B, NQ], BF16, tag="qT")
        mv_q = small.tile([KD, 2], F32, tag="mv_q")
        nc.vector.bn_aggr(out=mv_q, in_=st_q)
        s_q, t_q = bn_scale_shift(mv_q, qgb_sb, KD, "q")
        for b in range(B):
            nc.vector.tensor_scalar(out=qT[0:KD, b, :], in0=y_q[:, b, :],
                                    scalar1=s_q, scalar2=t_q,
                                    op0=mult, op1=add)
        mv_kv = small.tile([KVP, 2], F32, tag="mv_kv")
        nc.vector.bn_aggr(out=mv_kv, in_=st_kv)
        s_kv, t_kv = bn_scale_shift(mv_kv, kvgb_sb, KVP, "kv")
        # normalize only the k-tiles the loop touches first; the rest is
        # emitted inside the loop where the DVE has slack
        KHEAD = 6 * 128
        for b in range(B):
            nc.vector.tensor_scalar(out=kT[0:KD, b, 0:KHEAD],
                                    in0=y_kv[0:KD, b, 0:KHEAD],
                                    scalar1=s_kv[0:KD], scalar2=t_kv[0:KD],
                                    op0=mult, op1=add)

        def kT_norm_tail():
            for b in range(B):
                nc.vector.tensor_scalar(out=kT[0:KD, b, KHEAD:NPAD],
                                        in0=y_kv[0:KD, b, KHEAD:NPAD],
                                        scalar1=s_kv[0:KD], scalar2=t_kv[0:KD],
                                        op0=mult, op1=add)

        # ------------------------- attention -------------------------
        # qc-outer so each exp(bias) tile is DMA'd once and shared by both
        # batches. Each chunk's accumulators are parked to SBUF immediately
        # (freeing PSUM), but the softmax-divide/hardswish math is deferred
        # into the next chunk's g-loop so the DVE queue never stalls the ACT
        # exp pipeline at chunk boundaries.
        hsT = big.tile([DV, B, NQ], BF16, tag="hsT")
        hs_bounce = dram.tile([NQC, DV, B * QC], BF16, tag="hs_bounce")
        hs_all = dram.tile([NQC, H * DV, B * QC], BF16, tag="hs_all")
        y_p = big.tile([OC, B * NQ], BF16, tag="ypf")
        st_p = small.tile([OC, NQC * B, 6], F32, tag="st_p")

        # v's BN affine, shifted to partitions 0:DV for the drain epilogue
        sv01 = const.tile([DV, 1], F32, tag="sv01")
        tv01 = const.tile([DV, 1], F32, tag="tv01")
        tv3 = const.tile([DV, 1], F32, tag="tv3")
        nc.sync.dma_start(out=sv01, in_=s_kv[32:KVP])
        nc.sync.dma_start(out=tv01, in_=t_kv[32:KVP])
        nc.vector.tensor_scalar(out=tv3, in0=tv01, scalar1=3.0, scalar2=None,
                                op0=add)

        def drain_math(qc, av_sbs, bs=(0, 1)):
            for b in bs:
                av = av_sbs[b]
                rec = drain.tile([1, QC], F32, tag="rec")
                nc.vector.reciprocal(out=rec, in_=av[DV:DV + 1, :])
                # broadcast 1/denominator across the 32 value rows on gpsimd
                recb = drain.tile([DV, QC], F32, tag="recb")
                nc.gpsimd.partition_broadcast(recb, rec)
                # fold v's BatchNorm into the divide: weights sum to 1, so
                # the normalized context is s_v * (raw context) + t_v; t_v
                # rides in via the clamp offset and the final fused mult
                xo = drain.tile([DV, QC], F32, tag="xo")
                nc.vector.scalar_tensor_tensor(out=xo, in0=av[0:DV, :],
                                               scalar=sv01, in1=recb,
                                               op0=mult, op1=mult)
                r3 = drain.tile([DV, QC], F32, tag="r3")
                nc.vector.tensor_scalar(out=r3, in0=xo, scalar1=tv3,
                                        scalar2=0.0, op0=add,
                                        op1=mybir.AluOpType.max)
                nc.vector.tensor_scalar(out=r3, in0=r3, scalar1=6.0,
                                        scalar2=1.0 / 6.0, op0=amin, op1=mult)
                nc.vector.scalar_tensor_tensor(
                    out=hsT[:, b, bass.ts(qc, QC)], in0=xo, scalar=tv01,
                    in1=r3, op0=add, op1=mult)
            if bs[-1] == 1:
                nc.sync.dma_start(
                    out=hs_bounce[qc].rearrange("d (b q) -> d b q", b=B),
                    in_=hsT[:, :, bass.ts(qc, QC)])
                nc.gpsimd.collective_compute(
                    "AllGather", mybir.AluOpType.bypass,
                    replica_groups=[list(range(NCORES))],
                    ins=[hs_bounce[qc].opt()],
                    outs=[hs_all[qc].opt()])

        pending = None
        for qc in range(NQC):
            avs = []
            for b in range(B):
                av_t = psB.tile([DV + 1, QC], F32, tag="ps_small")
                avs.append(av_t)
            prev_sps = None
            for g in range(NGRP):
                eb = ebpool.tile([128, GRP, QC], BF16, tag="eb")
                nc.sync.dma_start(
                    out=eb,
                    in_=ebT[qc, g].rearrange("p (i q) -> p i q", i=GRP))
                cur_sps = []
                for b in range(B):
                    qk = psA.tile([128, GRP, 512], F32, tag="qk")
                    for i in range(GRP):
                        j = g * GRP + i
                        nc.tensor.matmul(qk[:, i, 0:QC],
                                         kT[:, b, bass.ts(j, 128)],
                                         qT[:, b, bass.ts(qc, QC)],
                                         start=True, stop=True)
                    sp = spool.tile([128, GRP, QC], BF16, tag="sp")
                    nc.scalar.activation(out=sp, in_=qk[:, :, 0:QC],
                                         func=Act.Exp, scale=SCALE)
                    nc.vector.tensor_mul(sp, sp, eb)
                    cur_sps.append(sp)
                # attn@v for the PREVIOUS group: keeps the PE queue stocked
                # with QKs so a slow mult never starves the exp pipeline
                if prev_sps is not None:
                    for b in range(B):
                        for i in range(GRP):
                            j = (g - 1) * GRP + i
                            nc.tensor.matmul(avs[b], v_aug[:, b, j, :],
                                             prev_sps[b][:, i, :],
                                             start=(j == 0), stop=False,
                                             skip_group_check=True)
                prev_sps = cur_sps
                if g == 0 and qc == 0:
                    kT_norm_tail()
                if g == 1 and pending is not None:
                    drain_math(*pending, bs=(0,))
                if g == 3 and pending is not None:
                    drain_math(*pending, bs=(1,))
                    pending = None
            # final group's attn@v
            for b in range(B):
                for i in range(GRP):
                    j = (NGRP - 1) * GRP + i
                    nc.tensor.matmul(avs[b], v_aug[:, b, j, :],
                                     prev_sps[b][:, i, :],
                                     start=(j == 0), stop=(j == KT - 1),
                                     skip_group_check=True)
            # park the accumulators in SBUF right away so the PSUM slots
            # free for the next chunk
            av_sbs = []
            for b in range(B):
                av_sb = drain.tile([DV + 1, QC], F32, tag="av_sb")
                nc.vector.tensor_copy(av_sb, avs[b])
                av_sbs.append(av_sb)
            pending = (qc, av_sbs)
        drain_math(*pending)

        # --------------------- projection (chunked) ---------------------
        for qc in range(NQC):
            hsall_sb = rspool.tile([128, 2, B * QC], BF16, tag="hsall")
            for c in range(2):
                nc.sync.dma_start(out=hsall_sb[:, c, :],
                                  in_=hs_all[qc, bass.ts(c, 128), :])
            for b in range(B):
                ps = psB.tile([OC, QC], F32, tag="ps_small")
                for c in range(2):
                    nc.tensor.matmul(ps, wp_sb[:, c, :],
                                     hsall_sb[:, c, bass.ds(b * QC, QC)],
                                     start=(c == 0), stop=(c == 1))
                nc.vector.bn_stats(out=st_p[:, qc * B + b, :], in_=ps)
                nc.vector.tensor_copy(
                    y_p[:, bass.ds(b * NQ + qc * QC, QC)], ps)
        mv_p = small.tile([OC, 2], F32, tag="mv_p")
        nc.vector.bn_aggr(out=mv_p, in_=st_p)
        s_p, t_p = bn_scale_shift(mv_p, pgb_sb, OC, "p")
        y_out = big.tile([OC, B * NQ], BF16, tag="yout")
        HN = B * NQ // 2
        for h2 in range(2):
            nc.vector.tensor_scalar(out=y_out[:, bass.ts(h2, HN)],
                                    in0=y_p[:, bass.ts(h2, HN)],
                                    scalar1=s_p, scalar2=t_p,
                                    op0=mult, op1=add)
            nc.sync.dma_start(out=yT[:, bass.ts(h2, HN)],
                              in_=y_out[:, bass.ts(h2, HN)])

    nc.finalize()
    return nc


def _prep_inputs(x, kv_w, kv_g, kv_b, q_w, q_g, q_b, proj_w, proj_g, proj_b,
                 bias_table, bias_idxs):
    """Host-side sharding/layout prep. Returns list of 8 per-core input maps."""
    x = np.asarray(x, np.float32)
    # x^T padded: (B, 2, 128, NPAD)
    xt = np.zeros((B, 2, 128, NPAD), np.float32)
    xTt = x.transpose(0, 2, 1)  # (B, 256, N)
    xt[:, :, :, :N] = xTt.reshape(B, 2, 128, N)
    xt = xt.astype(bf16)

    # exp(bias) tables per head, padded-k zeroed, laid out (NQC, NGRP, 128, GRP*QC)
    rank2 = np.asarray(bias_idxs)[0].reshape(ROW, COL)  # (dr, dc) -> id
    table2 = np.asarray(bias_table, np.float32)[:, rank2]  # (H, 63, 84)
    eb2 = np.exp(table2)
    kk = np.arange(N)
    qq = np.arange(NQ)
    DRm = np.abs(kk[:, None] // COL - 2 * (qq[None, :] // COL_))
    DCm = np.abs(kk[:, None] % COL - 2 * (qq[None, :] % COL_))

    in_maps = []
    for h in range(H):
        ebf = np.zeros((NPAD, NQ), np.float32)
        ebf[:N] = eb2[h][DRm, DCm]
        # (NPAD, NQ) -> (NQC, NGRP, 128, GRP*QC)
        ebl = (ebf.reshape(NGRP, GRP, 128, NQC, QC)
               .transpose(3, 0, 2, 1, 4)
               .reshape(NQC, NGRP, 128, GRP * QC)).astype(bf16)
        sl = slice(h * HKV, (h + 1) * HKV)
        slq = slice(h * KD, (h + 1) * KD)
        slo = slice(h * OC, (h + 1) * OC)
        slv = slice(h * DV, (h + 1) * DV)
        # kv weights/gains padded to 64 rows: k at 0:16, v at 32:64
        wkv_pad = np.zeros((KVP, CIN), np.float32)
        wkv_pad[0:KD] = np.asarray(kv_w, np.float32)[sl][0:KD]
        wkv_pad[32:KVP] = np.asarray(kv_w, np.float32)[sl][KD:HKV]
        kvgb_pad = np.zeros((KVP, 2), np.float32)
        kvgb_pad[:, 0] = 1.0
        kvgb_pad[0:KD, 0] = np.asarray(kv_g, np.float32)[sl][0:KD]
        kvgb_pad[0:KD, 1] = np.asarray(kv_b, np.float32)[sl][0:KD]
        kvgb_pad[32:KVP, 0] = np.asarray(kv_g, np.float32)[sl][KD:HKV]
        kvgb_pad[32:KVP, 1] = np.asarray(kv_b, np.float32)[sl][KD:HKV]
        # packed weights: [2, 128, KVP(kv) + KD(q) + OC(proj)]
        wall = np.concatenate([
            wkv_pad.T.reshape(2, 128, KVP),
            np.asarray(q_w, np.float32)[slq].T.reshape(2, 128, KD),
            np.asarray(proj_w, np.float32)[slo].T.reshape(2, 128, OC)],
            axis=2)
        gball = np.zeros((KVP, 6), np.float32)
        gball[:, 0:2] = kvgb_pad
        gball[0:KD, 2] = np.asarray(q_g, np.float32)[slq]
        gball[0:KD, 3] = np.asarray(q_b, np.float32)[slq]
        gball[0:OC, 4] = np.asarray(proj_g, np.float32)[slo]
        gball[0:OC, 5] = np.asarray(proj_b, np.float32)[slo]
        in_maps.append({
            "xT": xt,
            "wAll": np.ascontiguousarray(wall).astype(bf16),
            "gbAll": np.ascontiguousarray(gball),
            "ebT": ebl,
        })
    return in_maps


def kernel(x, kv_w, kv_g, kv_b, q_w, q_g, q_b, proj_w, proj_g, proj_b,
           bias_table, bias_idxs, _trace=False):
    global LAST_EXEC_NS
    if "nc" not in _prog_cache:
        _prog_cache["nc"] = _build_program()
    nc = _prog_cache["nc"]
    in_maps = _prep_inputs(x, kv_w, kv_g, kv_b, q_w, q_g, q_b,
                           proj_w, proj_g, proj_b, bias_table, bias_idxs)
    res = run_bass_kernel_spmd(nc, in_maps, core_ids=list(range(NCORES)),
                               trace=_trace)
    LAST_EXEC_NS = res.exec_time_ns
    yts = [np.asarray(r["yT"]).astype(np.float32) for r in res.results]
    y = np.concatenate(yts, axis=0)                   # (512, B*NQ)
    return np.ascontiguousarray(
        y.T.reshape(B, NQ, H * OC).astype(np.float32))
